# revision 1
# baseline (speedup 1.0000x reference)
"""Trainium2 Bass kernel for GCNN message passing.

out[b] = relu((A @ x[b]) @ W + bias),  A sparse [N, N] from 800k edges.

Sharding (8 NeuronCores): core h owns output rows [h*6272, (h+1)*6272) for
ALL 4 batches. Host interleaves x into xcat[n] = x[:, n, :] (bf16,
[N, 4*128]) so ONE gather descriptor fetches a neighbor's features for all
4 batches at once (Q7 descriptor generation is the bottleneck resource, at
~8ns per gather index).

Device algorithm per core:
  Host pre-sorts the core's ~100k edges by destination row into 25
  row-blocks of 256 rows; within a block edges are split into "low"
  (col < 32768) / "high" groups so gather indices fit in int16, padded to
  uniform L / H tiles of 128 edges (col=0/val=0 padding).
  The scaled one-hot scatter matrices S[e, r] = (r == rl[e]) * val[e]
  ([128, 256] bf16 per edge-tile) are PREBUILT ON HOST and streamed in
  (DMA has headroom).
  Per row-block:
    - two dma_gather ops (bases xcat[0:], xcat[32768:]) fetch
      msgs [128(edge), T, 512] bf16; edge slot k -> partition k%128,
      tile k//128.
    - PE accumulates aggT_b[c, r] += msgs[:, t, b*128:+128].T @ S_t into
      PSUM [128, 4*256] f32 (segment sum via matmul accumulation).
    - aggT -> SBUF bf16, PE applies W (outT_b = W.T @ aggT_b) into a
      second PSUM tile, ACT applies relu(.+bias), batched DMA writes
      outT [4, 128, 6400] f32.
  Host transposes/concatenates the 8 per-core outputs.
"""
import sys

import numpy as np

try:  # concourse (Bass) lives in the trn repo
    import concourse  # noqa: F401
except ImportError:  # pragma: no cover
    sys.path.insert(0, "/opt/trn_rl_repo")

import ml_dtypes

B, N, E, C = 4, 50000, 800000, 128
LAST_RESULTS = None  # BassKernelResults of the most recent kernel() call
P = 128
BR = 256            # rows per block
RB = 25             # row-blocks per core (covers 6400 >= 6272 rows)
RH = 6272           # row stride between cores (8 * 6272 = 50176 >= N)
NCORES = 8
SPLIT = 32768       # low/high column split for int16 gather indices
OUT_DMA_BLKS = 4    # row-blocks per output DMA


def _pack_idx(vals, n_slots):
    """dma_gather int16 index layout: index k at [k % 16, k // 16],
    replicated to 128 partitions; 0-padded. -> [128, n_slots // 16]"""
    buf = np.zeros(n_slots, np.int16)
    buf[:len(vals)] = vals
    tile16 = buf.reshape(n_slots // 16, 16).T
    return np.tile(tile16, (8, 1))


def _preprocess(edge_row, edge_col, edge_vals):
    """Per-core gather-index tables and host-built S matrices.

    Returns (lowidx [8, RB, 128, 8L], highidx [8, RB, 128, 8H],
             smat [8, 128, RB*T*BR] bf16, L, H).
    Edge slot k of a block: partition k%128, tile k//128; slots < L*128
    low-group (col), the rest high-group (col - SPLIT).
    S tile t of block blk lives at smat[:, (blk*T+t)*BR:(blk*T+t+1)*BR].
    """
    per_core = []
    maxlow = maxhigh = 0
    for h in range(NCORES):
        lo, hi = h * RH, min((h + 1) * RH, N)
        m = (edge_row >= lo) & (edge_row < hi)
        r, c, v = edge_row[m] - lo, edge_col[m], edge_vals[m]
        is_high = c >= SPLIT
        order = np.lexsort((is_high, r // BR))
        r, c, v, is_high = r[order], c[order], v[order], is_high[order]
        blocks = []
        for blk in range(RB):
            sel = slice(*np.searchsorted(r // BR, [blk, blk + 1]))
            rb, cb, vb, hb = r[sel], c[sel], v[sel], is_high[sel]
            nlow = int((~hb).sum())
            blocks.append((rb, cb, vb, nlow))
            maxlow = max(maxlow, nlow)
            maxhigh = max(maxhigh, len(rb) - nlow)
        per_core.append(blocks)
    L = (maxlow + P - 1) // P
    H = (maxhigh + P - 1) // P
    T = L + H
    lowidx = np.zeros((NCORES, RB, P, 8 * L), np.int16)
    highidx = np.zeros((NCORES, RB, P, 8 * H), np.int16)
    smat = np.zeros((NCORES, P, RB * T * BR), ml_dtypes.bfloat16)
    iota = np.arange(BR, dtype=np.float32)
    for h in range(NCORES):
        for blk in range(RB):
            rb, cb, vb, nlow = per_core[h][blk]
            nh = len(rb) - nlow
            lowidx[h, blk] = _pack_idx(cb[:nlow], L * P)
            highidx[h, blk] = _pack_idx(cb[nlow:] - SPLIT, H * P)
            rr = np.zeros(T * P, np.float32)
            vv = np.zeros(T * P, np.float32)
            rr[:nlow] = (rb[:nlow] - blk * BR).astype(np.float32)
            vv[:nlow] = vb[:nlow]
            rr[L * P:L * P + nh] = (rb[nlow:] - blk * BR).astype(np.float32)
            vv[L * P:L * P + nh] = vb[nlow:]
            # S[e, r] for slot e=t*P+p -> smat[p, (blk*T+t)*BR + r]
            s_f32 = (iota[None, :] == rr[:, None]) * vv[:, None]  # [T*P, BR]
            smat[h, :, blk * T * BR:(blk + 1) * T * BR] = (
                s_f32.reshape(T, P, BR).transpose(1, 0, 2).reshape(P, T * BR)
                .astype(ml_dtypes.bfloat16))
    return lowidx, highidx, smat, L, H


def _build_program(L, H, n_blocks=RB, n_rows=N):
    import concourse.bacc as bacc
    import concourse.tile as tile
    from concourse import mybir
    from concourse._compat import get_trn_type

    T = L + H
    BC = B * C                       # 512 feature cols in xcat
    f32 = mybir.dt.float32
    bf16 = mybir.dt.bfloat16
    i16 = mybir.dt.int16
    nc = bacc.Bacc(get_trn_type() or "TRN2", target_bir_lowering=False)

    x_d = nc.dram_tensor("xcat", [n_rows, BC], bf16, kind="ExternalInput")
    lowidx_d = nc.dram_tensor("lowidx", [P, n_blocks * 8 * L], i16,
                              kind="ExternalInput")
    highidx_d = nc.dram_tensor("highidx", [P, n_blocks * 8 * H], i16,
                               kind="ExternalInput")
    smat_d = nc.dram_tensor("smat", [P, n_blocks * T * BR], bf16,
                            kind="ExternalInput")
    wt_d = nc.dram_tensor("wt", [C, C], bf16, kind="ExternalInput")
    bias_d = nc.dram_tensor("bias", [C, 1], f32, kind="ExternalInput")
    out_d = nc.dram_tensor("outT", [B, C, n_blocks * BR], f32,
                           kind="ExternalOutput")

    with tile.TileContext(nc) as tc:
        with (
            tc.tile_pool(name="const", bufs=1) as const_pool,
            tc.tile_pool(name="meta", bufs=1) as meta_pool,
            tc.tile_pool(name="msgs", bufs=3) as msgs_pool,
            tc.tile_pool(name="smat", bufs=3) as s_pool,
            tc.tile_pool(name="aggsb", bufs=2) as agg_pool,
            tc.tile_pool(name="ostage", bufs=2) as ostage_pool,
            tc.tile_pool(name="psum_agg", bufs=2, space="PSUM") as psA,
            tc.tile_pool(name="psum_out", bufs=2, space="PSUM") as psO,
        ):
            wt_sb = const_pool.tile([C, C], bf16)
            bias_sb = const_pool.tile([C, 1], f32)
            nc.sync.dma_start(out=wt_sb[:], in_=wt_d[:])
            nc.sync.dma_start(out=bias_sb[:], in_=bias_d[:])

            lowidx_sb = meta_pool.tile([P, n_blocks * 8 * L], i16)
            highidx_sb = meta_pool.tile([P, n_blocks * 8 * H], i16)
            nc.sync.dma_start(out=lowidx_sb[:], in_=lowidx_d[:])
            nc.sync.dma_start(out=highidx_sb[:], in_=highidx_d[:])

            ostage = None
            for blk in range(n_blocks):
                msgs = msgs_pool.tile([P, T, BC], bf16)
                nc.gpsimd.dma_gather(
                    out_ap=msgs[:, :L, :],
                    in_ap=x_d[:SPLIT, :],
                    idxs_ap=lowidx_sb[:, blk * 8 * L:(blk + 1) * 8 * L],
                    num_idxs=L * P,
                    num_idxs_reg=L * P,
                    elem_size=BC,
                    single_packet=False,
                )
                nc.gpsimd.dma_gather(
                    out_ap=msgs[:, L:, :],
                    in_ap=x_d[SPLIT:, :],
                    idxs_ap=highidx_sb[:, blk * 8 * H:(blk + 1) * 8 * H],
                    num_idxs=H * P,
                    num_idxs_reg=H * P,
                    elem_size=BC,
                    single_packet=False,
                )
                s_blk = s_pool.tile([P, T * BR], bf16)
                nc.sync.dma_start(
                    out=s_blk[:],
                    in_=smat_d[:, blk * T * BR:(blk + 1) * T * BR])
                aggT_ps = psA.tile([C, B * BR], f32)
                for bb in range(B):
                    for t in range(T):
                        nc.tensor.matmul(
                            out=aggT_ps[:, bb * BR:(bb + 1) * BR],
                            lhsT=msgs[:, t, bb * C:(bb + 1) * C],
                            rhs=s_blk[:, t * BR:(t + 1) * BR],
                            start=(t == 0), stop=(t == T - 1),
                        )
                aggT_sb = agg_pool.tile([C, B * BR], bf16)
                nc.vector.tensor_copy(out=aggT_sb[:], in_=aggT_ps[:])
                outT_ps = psO.tile([C, B * BR], f32)
                for bb in range(B):
                    nc.tensor.matmul(
                        out=outT_ps[:, bb * BR:(bb + 1) * BR],
                        lhsT=wt_sb[:],
                        rhs=aggT_sb[:, bb * BR:(bb + 1) * BR],
                        start=True, stop=True)
                if blk % OUT_DMA_BLKS == 0:
                    ostage = ostage_pool.tile([C, B, OUT_DMA_BLKS * BR], f32)
                o_off = (blk % OUT_DMA_BLKS) * BR
                for bb in range(B):
                    nc.scalar.activation(
                        out=ostage[:, bb, o_off:o_off + BR],
                        in_=outT_ps[:, bb * BR:(bb + 1) * BR],
                        func=mybir.ActivationFunctionType.Relu,
                        bias=bias_sb[:, :1], scale=1.0,
                    )
                if blk % OUT_DMA_BLKS == OUT_DMA_BLKS - 1 or blk == n_blocks - 1:
                    lo_blk = (blk // OUT_DMA_BLKS) * OUT_DMA_BLKS
                    width = (blk - lo_blk + 1) * BR
                    for bb in range(B):
                        nc.sync.dma_start(
                            out=out_d[bb, :, lo_blk * BR: lo_blk * BR + width],
                            in_=ostage[:, bb, :width],
                        )
    return nc


def _ensure_ntff_hook_importable():
    """bass_utils imports antenv.axon_hooks when BASS_TRACE is set; this
    image lacks that module. Provide a null hook so tracing degrades
    gracefully instead of crashing."""
    import types

    try:
        import antenv.axon_hooks  # noqa: F401
        return
    except ImportError:
        pass
    mod = types.ModuleType("antenv.axon_hooks")
    mod.get_axon_ntff_profile_hook = lambda: None
    mod.set_axon_ntff_profile_hook = lambda h: None
    sys.modules["antenv.axon_hooks"] = mod
    try:
        import antenv
        antenv.axon_hooks = mod
    except ImportError:
        pass


def kernel(x, edge_row, edge_col, edge_vals, W, b):
    _ensure_ntff_hook_importable()
    from concourse.bass_utils import run_bass_kernel_spmd

    x = np.asarray(x, np.float32)
    edge_row = np.asarray(edge_row, np.int32)
    edge_col = np.asarray(edge_col, np.int32)
    edge_vals = np.asarray(edge_vals, np.float32)
    W = np.asarray(W, np.float32)
    b = np.asarray(b, np.float32)

    lowidx, highidx, smat, L, H = _preprocess(edge_row, edge_col, edge_vals)
    nc = _build_program(L, H)
    nc.compile()

    # xcat[n] = x[:, n, :] flattened -> [N, 4*128] bf16
    xcat = np.ascontiguousarray(
        x.transpose(1, 0, 2).reshape(N, B * C)).astype(ml_dtypes.bfloat16)
    wt = W.astype(ml_dtypes.bfloat16)
    in_maps = []
    for h in range(NCORES):
        in_maps.append({
            "xcat": xcat,
            "lowidx": np.ascontiguousarray(
                lowidx[h].transpose(1, 0, 2).reshape(P, RB * 8 * L)),
            "highidx": np.ascontiguousarray(
                highidx[h].transpose(1, 0, 2).reshape(P, RB * 8 * H)),
            "smat": smat[h],
            "wt": wt,
            "bias": np.ascontiguousarray(b[:, None]),
        })

    res = run_bass_kernel_spmd(nc, in_maps, list(range(NCORES)))
    global LAST_RESULTS
    LAST_RESULTS = res

    out = np.empty((B, N, C), np.float32)
    for h in range(NCORES):
        lo, hi = h * RH, min((h + 1) * RH, N)
        o = res.results[h]["outT"]              # [B, C, RB*BR]
        for bb in range(B):
            out[bb, lo:hi] = o[bb].T[:hi - lo]
    return out



# revision 3
# speedup vs baseline: 1.0891x; 1.0891x over previous
"""Trainium2 Bass kernel for GCNN message passing.

out[b] = relu((A @ x[b]) @ W + bias),  A sparse [N, N] from 800k edges.

Sharding (8 NeuronCores): core h owns dest rows [h*6272, (h+1)*6272) for all
4 batches. Host interleaves x into xcat[n] = x[:, n, :] (bf16, [N, 4*128])
so one gather index fetches a neighbor's features for all 4 batches.

The gather (gpsimd dma_gather) costs ~3us/call + ~7ns/index of Q7 time and
is the bottleneck resource, so the design minimizes gather calls and index
count:
  - dest rows are processed in 25 blocks of 256 rows; gathers span 2 blocks
    (13 spans x 2 calls: one per int16 index half, col < / >= 32768).
  - within each (block, half), edges are deduplicated per 128-row sub-block
    (same col -> one gathered slot; its S column carries both rows' vals).
  - slot layout per (block, half): [0, PA) sub-A slots, [PA, PA+nb) sub-B
    slots, 0-padded to T*128. PA and T are max'd over the 8 cores so the
    SPMD instruction schedule is identical on every core.

Device per block (segment-sum via matmul, 128-row sub-blocks):
  - scatter: agg_ps[128 rows, 4*128 (b,c)] += S_tile[128 slot, 128 row].T
    @ msgs[128 slot, 512], accumulated over the block's tiles. S tiles are
    host-built bf16 and streamed (matmul order: sub A tiles then sub B).
  - drain per sub-block: DVE copy agg->SBUF bf16, 4x PE transpose ->
    aggT[128 c, 4*128 (b,r)], DVE copy, W GEMM (lhsT=W[c,o]), ACT
    relu(.+bias[o]) -> ostage bf16, batched DMA to outT [128 o, 49, 512].
Host transposes/concatenates per-core outputs and casts to f32.
"""
import sys

import numpy as np

try:  # concourse (Bass) lives in the trn repo
    import concourse  # noqa: F401
except ImportError:  # pragma: no cover
    sys.path.insert(0, "/opt/trn_rl_repo")

import ml_dtypes

B, N, E, C = 4, 50000, 800000, 128
LAST_RESULTS = None  # BassKernelResults of the most recent kernel() call
P = 128
BR = 256            # rows per block
SUB = 128           # rows per scatter sub-block
NBLK = 25           # row-blocks per core (covers 6400 >= 6272 rows)
RH = 6272           # row stride between cores (8 * 6272 = 50176 >= N)
NSUB = RH // SUB    # 49 valid sub-blocks per core
NCORES = 8
SPLIT = 32768       # low/high column split for int16 gather indices
SPAN = 2            # row-blocks per gather call pair
OUT_DMA_SUBS = 8    # sub-blocks per output DMA
BC = B * C          # 512 feature cols in xcat


def _pack_idx(vals, n_slots):
    """dma_gather int16 index layout: index k at [k % 16, k // 16],
    replicated to 128 partitions; 0-padded. -> [128, n_slots // 16]"""
    buf = np.zeros(n_slots, np.int16)
    buf[:len(vals)] = vals
    tile16 = buf.reshape(n_slots // 16, 16).T
    return np.tile(tile16, (8, 1))


def _preprocess(edge_row, edge_col, edge_vals):
    """Host: per-core gather index tables, S matrices, static schedule.

    Returns (sched, idx16 [8][128, TOTS//16], smat [8][128, NMM*128] bf16)
    where sched is a dict of static (per-block) structure shared by cores.
    """
    # --- bucket edges by (core, block, half, sub); collect unique cols ---
    # edges[(h, blk, half)] = (cols, rows_local256, vals, sub)
    buckets = {}
    for h in range(NCORES):
        lo, hi = h * RH, min((h + 1) * RH, N)
        m = (edge_row >= lo) & (edge_row < hi)
        r, c, v = edge_row[m] - lo, edge_col[m], edge_vals[m]
        blk = r // BR
        half = (c >= SPLIT).astype(np.int8)
        for b in range(NBLK):
            mb = blk == b
            for hf in range(2):
                mm = mb & (half == hf)
                buckets[(h, b, hf)] = (c[mm], r[mm] - b * BR, v[mm])

    # pass 1: per (blk, half) unique-col counts per sub -> static PA, T
    # uniq[(h, blk, half, sub)] = (unique_cols, edge_slot_pos, rows, vals)
    uniq = {}
    PA = np.zeros((NBLK, 2), np.int64)
    NBmax = np.zeros((NBLK, 2), np.int64)
    for (h, b, hf), (c, r, v) in buckets.items():
        sub = (r >= SUB).astype(np.int8)
        for s in range(2):
            ms = sub == s
            uc, inv = np.unique(c[ms], return_inverse=True)
            uniq[(h, b, hf, s)] = (uc, inv, r[ms] - s * SUB, v[ms])
            if s == 0:
                PA[b, hf] = max(PA[b, hf], len(uc))
            else:
                NBmax[b, hf] = max(NBmax[b, hf], len(uc))
    T = -(-(PA + NBmax) // P)          # tiles per (blk, half)
    assert np.all(T[:, :].sum(axis=1) > 0)

    # static matmul schedule per blk: list of (half, tile, sub), ordered by
    # (sub, half, tile) so each sub's PSUM accumulation group is consecutive
    sched_mm = []                      # [blk] -> list of (half, tile, sub)
    for b in range(NBLK):
        mm = []
        for s in range(2):
            for hf in range(2):
                if T[b, hf] == 0:
                    continue
                pa, t_all = int(PA[b, hf]), int(T[b, hf])
                tb, rem = divmod(pa, P)
                for t in range(t_all):
                    if s == 0:
                        if t < tb or (t == tb and rem > 0):
                            mm.append((hf, t, 0))
                    else:
                        if NBmax[b, hf] == 0:
                            continue
                        if t > tb or (t == tb and rem > 0) or \
                           (t == tb and rem == 0):
                            # rem == 0: tile tb starts sub B exactly
                            if t * P < pa + NBmax[b, hf]:
                                mm.append((hf, t, 1))
        # drop sub-B matmuls for the invalid trailing sub (blk 24 sub 1)
        if b * 2 + 1 >= NSUB:
            mm = [x for x in mm if x[2] == 0]
        sched_mm.append(mm)

    # tile offsets within a span's msgs buffer: [blk0.lo, blk1.lo,
    # blk0.hi, blk1.hi]
    moff = np.zeros((NBLK, 2), np.int64)
    span_tiles = []
    for j in range(0, NBLK, SPAN):
        blks = list(range(j, min(j + SPAN, NBLK)))
        off = 0
        for hf in range(2):
            for b in blks:
                moff[b, hf] = off
                off += T[b, hf]
        span_tiles.append(off)

    # pass 2: per-core idx tables and S matrices in static order
    nmm = [len(m) for m in sched_mm]
    idx16 = []
    smat = []
    for h in range(NCORES):
        idx_parts = []
        for j in range(0, NBLK, SPAN):
            blks = list(range(j, min(j + SPAN, NBLK)))
            for hf in range(2):
                vals = []
                for b in blks:
                    seg = np.zeros(int(T[b, hf]) * P, np.int64)
                    ua, _, _, _ = uniq[(h, b, hf, 0)]
                    ub, _, _, _ = uniq[(h, b, hf, 1)]
                    seg[:len(ua)] = ua
                    seg[PA[b, hf]:PA[b, hf] + len(ub)] = ub
                    if hf:
                        seg[:len(ua)] -= SPLIT
                        seg[PA[b, hf]:PA[b, hf] + len(ub)] -= SPLIT
                    vals.append(seg)
                v = np.concatenate(vals) if vals else np.zeros(0, np.int64)
                if len(v):
                    idx_parts.append(_pack_idx(v.astype(np.int16), len(v)))
        idx16.append(np.concatenate(idx_parts, axis=1))

        stiles = []
        for b in range(NBLK):
            # dense S per (half): [T*P, 256] then slice per matmul
            sfull = {}
            for hf in range(2):
                sf = np.zeros((int(T[b, hf]) * P, BR), np.float32)
                for s in range(2):
                    uc, inv, rr, vv = uniq[(h, b, hf, s)]
                    base = 0 if s == 0 else int(PA[b, hf])
                    np.add.at(sf, (base + inv, s * SUB + rr), vv)
                sfull[hf] = sf
            for hf, t, s in sched_mm[b]:
                stiles.append(
                    sfull[hf][t * P:(t + 1) * P, s * SUB:(s + 1) * SUB])
        sm = np.concatenate(stiles, axis=1) if stiles else \
            np.zeros((P, 0), np.float32)
        # stiles entries are [P, SUB]; concat along cols -> [P, nmm*SUB]
        smat.append(sm.astype(ml_dtypes.bfloat16))

    sched = dict(PA=PA, T=T, sched_mm=sched_mm, moff=moff,
                 span_tiles=span_tiles, nmm=nmm)
    return sched, idx16, smat


def _build_program(sched):
    import concourse.bacc as bacc
    import concourse.tile as tile
    from concourse import mybir
    from concourse._compat import get_trn_type

    T, PA = sched["T"], sched["PA"]
    sched_mm, moff = sched["sched_mm"], sched["moff"]
    span_tiles, nmm = sched["span_tiles"], sched["nmm"]
    tot_mm = int(np.sum(nmm))
    tot_idx16 = int(np.sum(T)) * P // 16

    f32 = mybir.dt.float32
    bf16 = mybir.dt.bfloat16
    i16 = mybir.dt.int16
    nc = bacc.Bacc(get_trn_type() or "TRN2", target_bir_lowering=False)

    x_d = nc.dram_tensor("xcat", [N, BC], bf16, kind="ExternalInput")
    idx_d = nc.dram_tensor("idx16", [P, tot_idx16], i16,
                           kind="ExternalInput")
    smat_d = nc.dram_tensor("smat", [P, tot_mm * SUB], bf16,
                            kind="ExternalInput")
    wt_d = nc.dram_tensor("wt", [C, C], bf16, kind="ExternalInput")
    bias_d = nc.dram_tensor("bias", [C, 1], f32, kind="ExternalInput")
    ident_d = nc.dram_tensor("ident", [P, P], bf16, kind="ExternalInput")
    out_d = nc.dram_tensor("outT", [C, NSUB, BC], bf16,
                           kind="ExternalOutput")

    with tile.TileContext(nc) as tc:
        with (
            tc.tile_pool(name="const", bufs=1) as const_pool,
            tc.tile_pool(name="meta", bufs=1) as meta_pool,
            tc.tile_pool(name="msgs", bufs=2) as msgs_pool,
            tc.tile_pool(name="smat", bufs=2) as s_pool,
            tc.tile_pool(name="aggsb", bufs=2) as agg_pool,
            tc.tile_pool(name="aggTsb", bufs=2) as aggT_pool,
            tc.tile_pool(name="ostage", bufs=2) as ostage_pool,
            tc.tile_pool(name="psum_agg", bufs=2, space="PSUM") as psA,
            tc.tile_pool(name="psum_tr", bufs=2, space="PSUM") as psT,
            tc.tile_pool(name="psum_out", bufs=2, space="PSUM") as psO,
        ):
            wt_sb = const_pool.tile([C, C], bf16)
            bias_sb = const_pool.tile([C, 1], f32)
            ident_sb = const_pool.tile([P, P], bf16)
            nc.sync.dma_start(out=wt_sb[:], in_=wt_d[:])
            nc.sync.dma_start(out=bias_sb[:], in_=bias_d[:])
            nc.sync.dma_start(out=ident_sb[:], in_=ident_d[:])

            idx_sb = meta_pool.tile([P, tot_idx16], i16)
            nc.sync.dma_start(out=idx_sb[:], in_=idx_d[:])

            ostage = None
            mm_base = 0          # running matmul index into smat
            idx_off = 0          # running idx16 column offset
            for j in range(0, NBLK, SPAN):
                blks = list(range(j, min(j + SPAN, NBLK)))
                ts = span_tiles[j // SPAN]
                msgs = msgs_pool.tile([P, ts, BC], bf16)
                # two gather calls: low half then high half
                tile_cursor = 0
                for hf in range(2):
                    nt = int(sum(T[b, hf] for b in blks))
                    if nt == 0:
                        continue
                    nidx = nt * P
                    nc.gpsimd.dma_gather(
                        out_ap=msgs[:, tile_cursor:tile_cursor + nt, :],
                        in_ap=x_d[:SPLIT, :] if hf == 0 else x_d[SPLIT:, :],
                        idxs_ap=idx_sb[:, idx_off:idx_off + nidx // 16],
                        num_idxs=nidx,
                        num_idxs_reg=nidx,
                        elem_size=BC,
                        single_packet=False,
                    )
                    tile_cursor += nt
                    idx_off += nidx // 16

                for b in blks:
                    s_sb = s_pool.tile([P, max(nmm[b], 1) * SUB], bf16)
                    nc.sync.dma_start(
                        out=s_sb[:, :nmm[b] * SUB],
                        in_=smat_d[:, mm_base * SUB:
                                   (mm_base + nmm[b]) * SUB])
                    agg_ps = psA.tile([P, 2 * BC], f32)
                    seen = {}
                    mmlist = sched_mm[b]
                    for k, (hf, t, s) in enumerate(mmlist):
                        first = s not in seen
                        seen[s] = True
                        last = all(x[2] != s for x in mmlist[k + 1:])
                        nc.tensor.matmul(
                            out=agg_ps[:, s * BC:(s + 1) * BC],
                            lhsT=s_sb[:, k * SUB:(k + 1) * SUB],
                            rhs=msgs[:, int(moff[b, hf]) + t, :],
                            start=first, stop=last,
                        )
                    mm_base += nmm[b]

                    for s in range(2):
                        g = b * 2 + s
                        if g >= NSUB:
                            continue
                        aggsb = agg_pool.tile([P, BC], bf16)
                        nc.vector.tensor_copy(
                            out=aggsb[:], in_=agg_ps[:, s * BC:(s + 1) * BC])
                        aggT_ps = psT.tile([P, BC], bf16)
                        for bb in range(B):
                            nc.tensor.transpose(
                                out=aggT_ps[:, bb * C:(bb + 1) * C],
                                in_=aggsb[:, bb * C:(bb + 1) * C],
                                identity=ident_sb[:],
                            )
                        aggTsb = aggT_pool.tile([P, BC], bf16)
                        nc.vector.tensor_copy(out=aggTsb[:], in_=aggT_ps[:])
                        outT_ps = psO.tile([P, BC], f32)
                        nc.tensor.matmul(
                            out=outT_ps[:], lhsT=wt_sb[:], rhs=aggTsb[:],
                            start=True, stop=True)
                        if g % OUT_DMA_SUBS == 0:
                            ostage = ostage_pool.tile(
                                [P, OUT_DMA_SUBS, BC], bf16)
                        nc.scalar.activation(
                            out=ostage[:, g % OUT_DMA_SUBS, :],
                            in_=outT_ps[:],
                            func=mybir.ActivationFunctionType.Relu,
                            bias=bias_sb[:, :1], scale=1.0,
                        )
                        if g % OUT_DMA_SUBS == OUT_DMA_SUBS - 1 or \
                           g == NSUB - 1:
                            glo = (g // OUT_DMA_SUBS) * OUT_DMA_SUBS
                            nsub = g - glo + 1
                            nc.sync.dma_start(
                                out=out_d[:, glo:glo + nsub, :],
                                in_=ostage[:, :nsub, :],
                            )
    return nc


def _ensure_ntff_hook_importable():
    """bass_utils imports antenv.axon_hooks when BASS_TRACE is set; this
    image lacks that module. Provide a null hook so tracing degrades
    gracefully instead of crashing."""
    import types

    try:
        import antenv.axon_hooks  # noqa: F401
        return
    except ImportError:
        pass
    mod = types.ModuleType("antenv.axon_hooks")
    mod.get_axon_ntff_profile_hook = lambda: None
    mod.set_axon_ntff_profile_hook = lambda h: None
    sys.modules["antenv.axon_hooks"] = mod
    try:
        import antenv
        antenv.axon_hooks = mod
    except ImportError:
        pass


def kernel(x, edge_row, edge_col, edge_vals, W, b):
    _ensure_ntff_hook_importable()
    from concourse.bass_utils import run_bass_kernel_spmd

    x = np.asarray(x, np.float32)
    edge_row = np.asarray(edge_row, np.int32)
    edge_col = np.asarray(edge_col, np.int32)
    edge_vals = np.asarray(edge_vals, np.float32)
    W = np.asarray(W, np.float32)
    b = np.asarray(b, np.float32)

    sched, idx16, smat = _preprocess(edge_row, edge_col, edge_vals)
    nc = _build_program(sched)
    nc.compile()

    # xcat[n] = x[:, n, :] flattened -> [N, 4*128] bf16
    xcat = np.ascontiguousarray(
        x.transpose(1, 0, 2).reshape(N, B * C)).astype(ml_dtypes.bfloat16)
    wt = W.astype(ml_dtypes.bfloat16)
    ident = np.eye(P, dtype=ml_dtypes.bfloat16)
    in_maps = []
    for h in range(NCORES):
        in_maps.append({
            "xcat": xcat,
            "idx16": idx16[h],
            "smat": smat[h],
            "wt": wt,
            "bias": np.ascontiguousarray(b[:, None]),
            "ident": ident,
        })

    res = run_bass_kernel_spmd(nc, in_maps, list(range(NCORES)))
    global LAST_RESULTS
    LAST_RESULTS = res

    out = np.empty((B, N, C), np.float32)
    for h in range(NCORES):
        lo, hi = h * RH, min((h + 1) * RH, N)
        o = res.results[h]["outT"].astype(np.float32)   # [C, NSUB, 4*128]
        # o[c, g, bb*128 + r] = out[bb, lo + g*128 + r, c]
        o = o.reshape(C, NSUB, B, SUB).transpose(2, 1, 3, 0) \
             .reshape(B, NSUB * SUB, C)
        out[:, lo:hi] = o[:, :hi - lo]
    return out
